# revision 23
# baseline (speedup 1.0000x reference)
"""Causal multi-head attention (B=4, S=2048, D=1024, H=16, hd=64) on 8 TRN2 cores.

Sharding: core c handles batch b = c//2 and heads [8*(c%2), 8*(c%2)+8).
Each core computes a partial output y_h @ Wo_rows for its 8 heads over its
batch; the host sums the two partials per batch.

v3 strategy (per core):
  - Host pre-casts x/Wqkv/Wo to bf16 (identical numerics to the previous
    on-chip casts since every consumer used bf16) -> input DMA halves.
  - xT loaded directly transposed from DRAM via DMA XBAR transpose
    (dma_start_transpose), one contiguous [128,512] tile per (dc, chunk):
    no PE transposes, no PSUM staging, no DVE copies for x at all.
  - qT/kT computed directly transposed (W-chunk stationary, xT moving);
    v natural with an appended ones column per head (v_aug) so PV also
    yields softmax denominators.
  - Scores transposed sT[k,q] = K @ Q^T, k-tiles in packs of 2 -> one
    2-bank PSUM strip, exp on ACT over contiguous runs (scale=1/8 folded
    in), causal triangle = 0/1 multiply (GpSimd) on diagonal blocks only.
  - Software pipelining: PV(pack i) emitted after scores(pack i+1), and
    dense qkv/proj matmul work is fed in ~2-matmul micro-chunks between
    packs (attention alone is ACT-bound; the in-order PE queue needs ready
    dense work at sub-head granularity).
  - Softmax reciprocal: each head's denominator row (PSUM partition 64) is
    copied into a [1,4096] strip (GpSimd), one SBUF->SBUF DMA repartitions
    it to [8,512], one DVE reciprocal_approx_fast, then a K=8 selector
    matmul per head broadcasts row h across 64 partitions. (Replaces the
    old per-head 4-DMA DRAM bounce.)
  - PSUM: 8 banks = psS(scores 2-bank)x2 + psY(y/bcast)x2 + psD(qkv/proj)x2.
"""

import numpy as np
from contextlib import ExitStack

import concourse.bass as bass
import concourse.tile as tile
from concourse import bacc, mybir
from concourse.bass import ts, ds
from concourse.bass_utils import run_bass_kernel_spmd
from concourse.masks import make_upper_triangular

S = 2048
D = 1024
NH = 8          # heads per core
HD = 64         # head dim
DSH = NH * HD   # 512, per-core shard width
P = 128
F32 = mybir.dt.float32
BF16 = mybir.dt.bfloat16
EXP = mybir.ActivationFunctionType.Exp
SCALE = 1.0 / 8.0  # 1/sqrt(HD)

N_STILES = S // P        # 16
N_QCHUNK = S // 512      # 4
N_DCHUNK = D // P        # 8
N_KCHUNK = DSH // P      # 4


class Feed:
    """Round-robin micro-chunk scheduler over generator work units."""

    def __init__(self):
        self.queue = []
        self.cur = None

    def add(self, gens):
        self.queue.extend(gens)

    def feed(self):
        while True:
            if self.cur is None:
                if not self.queue:
                    return False
                self.cur = self.queue.pop(0)()
            try:
                next(self.cur)
                return True
            except StopIteration:
                self.cur = None

    def drain(self):
        while self.feed():
            pass


def _emit(ctx: ExitStack, tc: tile.TileContext, x_ap, wq_ap, wk_ap, wv_ap, wo_ap, sel_ap, out_ap):
    nc = tc.nc

    const = ctx.enter_context(tc.tile_pool(name="const", bufs=1))
    trimask = const.tile([P, P], BF16, tag="trimask")
    make_upper_triangular(nc, trimask, val=1.0, diag=True)
    # sel[i, h*64+j] = (i == h): K=8 selector that broadcasts recip row h
    # across 64 partitions in one matmul. Loaded from host.
    sel = const.tile([NH, NH * HD], BF16, tag="sel")

    xT_pool = ctx.enter_context(tc.tile_pool(name="xT", bufs=1))
    w_pool = ctx.enter_context(tc.tile_pool(name="w", bufs=1))

    # xT[dc][sc]: contiguous [128, 512] tiles (XBAR-transposed loads need a
    # contiguous SBUF destination)
    xT = [
        [xT_pool.tile([P, 512], BF16, tag=f"xT{dc}_{sc}", name=f"xT{dc}_{sc}") for sc in range(4)]
        for dc in range(N_DCHUNK)
    ]
    wq = [w_pool.tile([P, DSH], BF16, tag=f"wq{dc}", name=f"wq{dc}") for dc in range(N_DCHUNK)]
    wk = [w_pool.tile([P, DSH], BF16, tag=f"wk{dc}", name=f"wk{dc}") for dc in range(N_DCHUNK)]
    wv = [w_pool.tile([P, DSH], BF16, tag=f"wv{dc}", name=f"wv{dc}") for dc in range(N_DCHUNK)]
    wo = [w_pool.tile([P, D], BF16, tag=f"wo{kc}", name=f"wo{kc}") for kc in range(N_KCHUNK)]

    qkT_pool = ctx.enter_context(tc.tile_pool(name="qkT", bufs=1))
    qT = [qkT_pool.tile([P, S], BF16, tag=f"qT{m}", name=f"qT{m}") for m in range(N_KCHUNK)]
    kT = [qkT_pool.tile([P, S], BF16, tag=f"kT{m}", name=f"kT{m}") for m in range(N_KCHUNK)]
    vaug_pool = ctx.enter_context(tc.tile_pool(name="vaug", bufs=1))
    vaug = [vaug_pool.tile([P, NH, HD + 1], BF16, tag=f"v{st}", name=f"v{st}") for st in range(N_STILES)]
    yT_pool = ctx.enter_context(tc.tile_pool(name="yTp", bufs=1))
    yT = [yT_pool.tile([P, S], BF16, tag=f"yT{kc}", name=f"yT{kc}") for kc in range(N_KCHUNK)]

    pT_pool = ctx.enter_context(tc.tile_pool(name="pT", bufs=3))
    ytmp_pool = ctx.enter_context(tc.tile_pool(name="ytp", bufs=9))
    r_pool = ctx.enter_context(tc.tile_pool(name="rp", bufs=2))
    o_pool = ctx.enter_context(tc.tile_pool(name="op", bufs=3))

    dram_pool = ctx.enter_context(tc.tile_pool(name="drp", bufs=2, space="DRAM"))
    psS = ctx.enter_context(tc.tile_pool(name="psS", bufs=2, space="PSUM"))
    psY = ctx.enter_context(tc.tile_pool(name="psY", bufs=2, space="PSUM"))
    psD = ctx.enter_context(tc.tile_pool(name="psD", bufs=2, space="PSUM"))

    def emit_xdma(sc):
        for dc in range(N_DCHUNK):
            nc.sync.dma_start_transpose(
                xT[dc][sc][:], x_ap[ds(sc * 512, 512), ts(dc, P)]
            )

    def emit_wdma():
        # issue on the scalar queue: keeps the sync queue free for the
        # XBAR transpose issues (which are ~1.3us each)
        for w_list, w_ap in ((wq, wq_ap), (wk, wk_ap), (wv, wv_ap)):
            for dc in range(N_DCHUNK):
                nc.scalar.dma_start(w_list[dc][:], w_ap[ts(dc, P), :])

    def emit_wodma():
        nc.gpsimd.dma_start(sel[:], sel_ap[:, :])
        for kc in range(N_KCHUNK):
            nc.gpsimd.dma_start(wo[kc][:], wo_ap[ts(kc, P), :])

    # ---- filler units: generators yielding every ~2 matmuls ----
    def qk_unit(sc, w_list, o_list, m):
        def gen():
            pc = psD.tile([P, 512], F32, tag="pd")
            for dc in range(N_DCHUNK):
                nc.tensor.matmul(
                    pc[:],
                    lhsT=w_list[dc][:, ts(m, P)],
                    rhs=xT[dc][sc][:],
                    start=(dc == 0),
                    stop=(dc == N_DCHUNK - 1),
                )
                if dc < N_DCHUNK - 1:
                    yield
            nc.vector.tensor_copy(o_list[m][:, ts(sc, 512)], pc[:])
        return gen

    def v_unit(sc, j):
        def gen():
            st = sc * 4 + j
            pc = psD.tile([P, 512], F32, tag="pd")
            for dc in range(N_DCHUNK):
                nc.tensor.matmul(
                    pc[:],
                    lhsT=xT[dc][sc][:, ts(j, P)],
                    rhs=wv[dc][:],
                    start=(dc == 0),
                    stop=(dc == N_DCHUNK - 1),
                )
                if dc < N_DCHUNK - 1:
                    yield
            nc.vector.tensor_copy(
                vaug[st][:, :, 0:HD],
                pc[:].rearrange("p (h d) -> p h d", h=NH),
            )
            nc.gpsimd.memset(vaug[st][:, :, HD : HD + 1], 1.0)
        return gen

    def proj_unit(st):
        # yields after every matmul: fine-grained chunks so the feed supply
        # can be rationed across all of the last attention chunk's packs
        def gen():
            ot = o_pool.tile([P, D], F32, tag="o")
            for ncol in range(2):
                po = psD.tile([P, 512], F32, tag="pd")
                for kc in range(N_KCHUNK):
                    nc.tensor.matmul(
                        po[:],
                        lhsT=yT[kc][:, ts(st, P)],
                        rhs=wo[kc][:, ts(ncol, 512)],
                        start=(kc == 0),
                        stop=(kc == N_KCHUNK - 1),
                    )
                    if kc < N_KCHUNK - 1:
                        yield
                nc.vector.tensor_copy(ot[:, ts(ncol, 512)], po[:])
                if ncol == 0:
                    yield
            nc.gpsimd.dma_start(out_ap[ts(st, P), :], ot[:])
        return gen

    def qkv_units(sc, q_first=False):
        units = []
        if q_first:
            # all q before k: lets chunk-0 projections start as soon as
            # x chunk 0 + wq have landed, before wk/wv arrive
            for m in range(N_KCHUNK):
                units.append(qk_unit(sc, wq, qT, m))
            for m in range(N_KCHUNK):
                units.append(qk_unit(sc, wk, kT, m))
        else:
            for m in range(N_KCHUNK):
                units.append(qk_unit(sc, wq, qT, m))
                units.append(qk_unit(sc, wk, kT, m))
        for j in range(4):
            units.append(v_unit(sc, j))
        return units

    feed = Feed()

    # ---- attention for one q-chunk, dense work fed between packs ----
    def emit_attn(qc, units=(), nres=0):
        units = list(units)
        if nres:
            feed.add(units[:-nres])
        else:
            feed.add(units)
        q0 = qc * 512
        n_kt = qc * 4 + 4
        diag0 = qc * 4
        dall = r_pool.tile([1, NH * 512], F32, tag="da")
        yts = []
        for h in range(NH):
            tile_i = h // 2
            row0 = (h % 2) * HD
            kT_h = kT[tile_i][row0 : row0 + HD, :]
            qT_h = qT[tile_i][row0 : row0 + HD, :]
            psum_y = psY.tile([P, 512], F32, tag="py", name=f"py{qc}_{h}")
            prev = None
            for p0 in range(0, n_kt, 2):
                pack = [p0, p0 + 1]
                pss = psS.tile([P, 1024], F32, tag="ps", name=f"ps{qc}_{h}_{p0}")
                pT = pT_pool.tile([P, 1024], BF16, tag="pT")
                offs = {}
                for idx, kt in enumerate(pack):
                    w = 512 if kt < diag0 else 512 - 128 * (kt - diag0)
                    off = idx * 512
                    qoff = q0 + (512 - w)
                    nc.tensor.matmul(
                        pss[:, ds(off, w)],
                        lhsT=kT_h[:, ts(kt, P)],
                        rhs=qT_h[:, ds(qoff, w)],
                        start=True,
                        stop=True,
                    )
                    offs[kt] = (off, w)
                # exp over contiguous runs
                runs = []
                for kt in pack:
                    off, w = offs[kt]
                    if runs and runs[-1][1] == off:
                        runs[-1][1] = off + w
                    else:
                        runs.append([off, off + w])
                for r0, r1 in runs:
                    nc.scalar.activation(
                        pT[:, ds(r0, r1 - r0)], pss[:, ds(r0, r1 - r0)], EXP, scale=SCALE
                    )
                for kt in pack:
                    off, w = offs[kt]
                    if kt >= diag0:
                        nc.gpsimd.tensor_mul(
                            pT[:, ds(off, P)], pT[:, ds(off, P)], trimask[:]
                        )
                if prev is not None:
                    for pkt, ppT, poff, pw in prev:
                        nc.tensor.matmul(
                            psum_y[0 : HD + 1, ds(512 - pw, pw)],
                            lhsT=vaug[pkt][:, h, :],
                            rhs=ppT[:, ds(poff, pw)],
                            start=(pkt == 0),
                            stop=False,
                            skip_group_check=True,
                        )
                prev = [(kt, pT, offs[kt][0], offs[kt][1]) for kt in pack]
                feed.feed()
            for i, (pkt, ppT, poff, pw) in enumerate(prev):
                nc.tensor.matmul(
                    psum_y[0 : HD + 1, ds(512 - pw, pw)],
                    lhsT=vaug[pkt][:, h, :],
                    rhs=ppT[:, ds(poff, pw)],
                    start=(pkt == 0),
                    stop=(pkt == n_kt - 1),
                    skip_group_check=True,
                )
            yt = ytmp_pool.tile([HD, 512], BF16, tag="yt")
            nc.vector.tensor_copy(yt[:], psum_y[0:HD, :])
            yts.append(yt)
            # den row (PSUM partition 64) -> strip segment (partition 0)
            nc.vector.tensor_copy(dall[0:1, ds(h * 512, 512)], psum_y[HD : HD + 1, :])
            feed.feed()
        # repartition [1,4096] -> [8,512] via DRAM bounce, batch the reciprocal
        dscr = dram_pool.tile([NH * 512], F32, tag="ds")
        nc.scalar.dma_start(dscr[:], dall[0:1, :])
        stage = r_pool.tile([NH, 512], F32, tag="st")
        nc.scalar.dma_start(stage[:], dscr[:].rearrange("(p c) -> p c", p=NH))
        rec = r_pool.tile([NH, 512], F32, tag="rc")
        nc.vector.reciprocal_approx_fast(rec[:], stage[:])
        rbf = r_pool.tile([NH, 512], BF16, tag="rb")
        nc.gpsimd.tensor_copy(rbf[:], rec[:])
        if nres:
            feed.add(units[-nres:])
        feed.feed()
        feed.feed()
        for hp in range(NH // 2):
            # one K=8 selector matmul broadcasts recip rows for a head PAIR:
            # psr rows 0:64 = head 2*hp, rows 64:128 = head 2*hp+1
            tile_i = hp
            psr = psY.tile([P, 512], F32, tag="py", name=f"pr{qc}_{hp}")
            nc.tensor.matmul(
                psr[:], lhsT=sel[:, ds(2 * hp * HD, P)], rhs=rbf[:], start=True, stop=True
            )
            rfull = r_pool.tile([P, 512], BF16, tag="rf")
            nc.vector.tensor_copy(rfull[:], psr[:])
            nc.vector.tensor_mul(
                yT[tile_i][0:HD, ts(qc, 512)], yts[2 * hp][:], rfull[0:HD, :]
            )
            nc.vector.tensor_mul(
                yT[tile_i][HD:P, ts(qc, 512)], yts[2 * hp + 1][:], rfull[HD:P, :]
            )
            feed.feed()

    # ---- emission schedule ----
    emit_xdma(0)
    emit_wdma()
    emit_xdma(1)
    for g in qkv_units(0, q_first=True):  # chunk 0 qkv inline (nothing to overlap)
        for _ in g():
            pass
    emit_wodma()
    emit_xdma(2)

    emit_attn(0, qkv_units(1), nres=2)
    feed.drain()               # qkv(1) must complete before attn(1)
    emit_xdma(3)

    emit_attn(1, qkv_units(2), nres=2)
    feed.drain()

    emit_attn(2, qkv_units(3), nres=2)
    feed.drain()

    emit_attn(3, [proj_unit(st) for st in range(0, 12)], nres=4)
    feed.drain()

    for g in [proj_unit(st) for st in range(12, 16)]:
        for _ in g():
            pass


def build_nc():
    nc = bacc.Bacc("TRN2", target_bir_lowering=False, debug=False)
    x_ap = nc.dram_tensor("x", [S, D], BF16, kind="ExternalInput").ap()
    wq_ap = nc.dram_tensor("wq", [D, DSH], BF16, kind="ExternalInput").ap()
    wk_ap = nc.dram_tensor("wk", [D, DSH], BF16, kind="ExternalInput").ap()
    wv_ap = nc.dram_tensor("wv", [D, DSH], BF16, kind="ExternalInput").ap()
    wo_ap = nc.dram_tensor("wo", [DSH, D], BF16, kind="ExternalInput").ap()
    sel_ap = nc.dram_tensor("sel", [NH, NH * HD], BF16, kind="ExternalInput").ap()
    out_ap = nc.dram_tensor("out", [S, D], F32, kind="ExternalOutput").ap()
    with tile.TileContext(nc) as tc:
        with ExitStack() as ctx:
            _emit(ctx, tc, x_ap, wq_ap, wk_ap, wv_ap, wo_ap, sel_ap, out_ap)
    nc.compile()
    return nc


_NC = None


def _get_nc():
    global _NC
    if _NC is None:
        _NC = build_nc()
    return _NC


def _bf16(a):
    import ml_dtypes

    return np.ascontiguousarray(a.astype(ml_dtypes.bfloat16))


def make_in_maps(x, Wqkv, Wo):
    Wq, Wk, Wv = Wqkv[:, 0:D], Wqkv[:, D : 2 * D], Wqkv[:, 2 * D : 3 * D]
    sel = np.kron(np.eye(NH, dtype=np.float32), np.ones((1, HD), np.float32))
    in_maps = []
    for c in range(8):
        b, hh = c // 2, c % 2
        cs = slice(hh * DSH, (hh + 1) * DSH)
        in_maps.append(
            {
                "x": _bf16(x[b]),
                "wq": _bf16(Wq[:, cs]),
                "wk": _bf16(Wk[:, cs]),
                "wv": _bf16(Wv[:, cs]),
                "wo": _bf16(Wo[cs, :]),
                "sel": _bf16(sel),
            }
        )
    return in_maps


def kernel(x, Wqkv, Wo, trace=False):
    x = np.asarray(x)
    Wqkv = np.asarray(Wqkv)
    Wo = np.asarray(Wo)
    nc = _get_nc()
    res = run_bass_kernel_spmd(nc, make_in_maps(x, Wqkv, Wo), list(range(8)), trace=trace)
    out = np.empty((4, S, D), np.float32)
    for b in range(4):
        out[b] = res.results[2 * b]["out"] + res.results[2 * b + 1]["out"]
    if trace:
        kernel.last_exec_time_ns = res.exec_time_ns
        kernel.last_results = res
    return out


# revision 24
# speedup vs baseline: 1.0332x; 1.0332x over previous
"""Causal multi-head attention (B=4, S=2048, D=1024, H=16, hd=64) on 8 TRN2 cores.

Sharding: core c handles batch b = c//2 and heads [8*(c%2), 8*(c%2)+8).
Each core computes a partial output y_h @ Wo_rows for its 8 heads over its
batch; the host sums the two partials per batch.

v3 strategy (per core):
  - Host pre-casts x/Wqkv/Wo to bf16 (identical numerics to the previous
    on-chip casts since every consumer used bf16) -> input DMA halves.
  - xT loaded directly transposed from DRAM via DMA XBAR transpose
    (dma_start_transpose), one contiguous [128,512] tile per (dc, chunk):
    no PE transposes, no PSUM staging, no DVE copies for x at all.
  - qT/kT computed directly transposed (W-chunk stationary, xT moving);
    v natural with an appended ones column per head (v_aug) so PV also
    yields softmax denominators.
  - Scores transposed sT[k,q] = K @ Q^T, k-tiles in packs of 2 -> one
    2-bank PSUM strip, exp on ACT over contiguous runs (scale=1/8 folded
    in), causal triangle = 0/1 multiply (GpSimd) on diagonal blocks only.
  - Software pipelining: PV(pack i) emitted after scores(pack i+1), and
    dense qkv/proj matmul work is fed in ~2-matmul micro-chunks between
    packs (attention alone is ACT-bound; the in-order PE queue needs ready
    dense work at sub-head granularity).
  - Softmax reciprocal: each head's denominator row (PSUM partition 64) is
    copied into a [1,4096] strip (GpSimd), one SBUF->SBUF DMA repartitions
    it to [8,512], one DVE reciprocal_approx_fast, then a K=8 selector
    matmul per head broadcasts row h across 64 partitions. (Replaces the
    old per-head 4-DMA DRAM bounce.)
  - PSUM: 8 banks = psS(scores 2-bank)x2 + psY(y/bcast)x2 + psD(qkv/proj)x2.
"""

import numpy as np
from contextlib import ExitStack

import concourse.bass as bass
import concourse.tile as tile
from concourse import bacc, mybir
from concourse.bass import ts, ds
from concourse.bass_utils import run_bass_kernel_spmd
from concourse.masks import make_upper_triangular

S = 2048
D = 1024
NH = 8          # heads per core
HD = 64         # head dim
DSH = NH * HD   # 512, per-core shard width
P = 128
F32 = mybir.dt.float32
BF16 = mybir.dt.bfloat16
EXP = mybir.ActivationFunctionType.Exp
SCALE = 1.0 / 8.0  # 1/sqrt(HD)

N_STILES = S // P        # 16
N_QCHUNK = S // 512      # 4
N_DCHUNK = D // P        # 8
N_KCHUNK = DSH // P      # 4


class Feed:
    """Round-robin micro-chunk scheduler over generator work units."""

    def __init__(self):
        self.queue = []
        self.cur = None

    def add(self, gens):
        self.queue.extend(gens)

    def feed(self):
        while True:
            if self.cur is None:
                if not self.queue:
                    return False
                self.cur = self.queue.pop(0)()
            try:
                next(self.cur)
                return True
            except StopIteration:
                self.cur = None

    def drain(self):
        while self.feed():
            pass


def _emit(ctx: ExitStack, tc: tile.TileContext, x_ap, wq_ap, wk_ap, wv_ap, wo_ap, sel_ap, out_ap):
    nc = tc.nc

    const = ctx.enter_context(tc.tile_pool(name="const", bufs=1))
    trimask = const.tile([P, P], BF16, tag="trimask")
    make_upper_triangular(nc, trimask, val=1.0, diag=True)
    # sel[i, h*64+j] = (i == h): K=8 selector that broadcasts recip row h
    # across 64 partitions in one matmul. Loaded from host.
    sel = const.tile([NH, NH * HD], BF16, tag="sel")

    xT_pool = ctx.enter_context(tc.tile_pool(name="xT", bufs=1))
    w_pool = ctx.enter_context(tc.tile_pool(name="w", bufs=1))

    # xT[dc][sc]: contiguous [128, 512] tiles (XBAR-transposed loads need a
    # contiguous SBUF destination)
    xT = [
        [xT_pool.tile([P, 512], BF16, tag=f"xT{dc}_{sc}", name=f"xT{dc}_{sc}") for sc in range(4)]
        for dc in range(N_DCHUNK)
    ]
    wq = [w_pool.tile([P, DSH], BF16, tag=f"wq{dc}", name=f"wq{dc}") for dc in range(N_DCHUNK)]
    wk = [w_pool.tile([P, DSH], BF16, tag=f"wk{dc}", name=f"wk{dc}") for dc in range(N_DCHUNK)]
    wv = [w_pool.tile([P, DSH], BF16, tag=f"wv{dc}", name=f"wv{dc}") for dc in range(N_DCHUNK)]
    wo = [w_pool.tile([P, D], BF16, tag=f"wo{kc}", name=f"wo{kc}") for kc in range(N_KCHUNK)]

    qkT_pool = ctx.enter_context(tc.tile_pool(name="qkT", bufs=1))
    qT = [qkT_pool.tile([P, S], BF16, tag=f"qT{m}", name=f"qT{m}") for m in range(N_KCHUNK)]
    kT = [qkT_pool.tile([P, S], BF16, tag=f"kT{m}", name=f"kT{m}") for m in range(N_KCHUNK)]
    vaug_pool = ctx.enter_context(tc.tile_pool(name="vaug", bufs=1))
    vaug = [vaug_pool.tile([P, NH, HD + 1], BF16, tag=f"v{st}", name=f"v{st}") for st in range(N_STILES)]
    yT_pool = ctx.enter_context(tc.tile_pool(name="yTp", bufs=1))
    yT = [yT_pool.tile([P, S], BF16, tag=f"yT{kc}", name=f"yT{kc}") for kc in range(N_KCHUNK)]

    pT_pool = ctx.enter_context(tc.tile_pool(name="pT", bufs=3))
    ytmp_pool = ctx.enter_context(tc.tile_pool(name="ytp", bufs=5))
    r_pool = ctx.enter_context(tc.tile_pool(name="rp", bufs=2))
    o_pool = ctx.enter_context(tc.tile_pool(name="op", bufs=3))

    dram_pool = ctx.enter_context(tc.tile_pool(name="drp", bufs=2, space="DRAM"))
    psS = ctx.enter_context(tc.tile_pool(name="psS", bufs=2, space="PSUM"))
    psY = ctx.enter_context(tc.tile_pool(name="psY", bufs=2, space="PSUM"))
    psD = ctx.enter_context(tc.tile_pool(name="psD", bufs=2, space="PSUM"))

    def emit_xdma(sc):
        for dc in range(N_DCHUNK):
            nc.sync.dma_start_transpose(
                xT[dc][sc][:], x_ap[ds(sc * 512, 512), ts(dc, P)]
            )

    def emit_wdma():
        # issue on the scalar queue: keeps the sync queue free for the
        # XBAR transpose issues (which are ~1.3us each)
        for w_list, w_ap in ((wq, wq_ap), (wk, wk_ap), (wv, wv_ap)):
            for dc in range(N_DCHUNK):
                nc.scalar.dma_start(w_list[dc][:], w_ap[ts(dc, P), :])

    def emit_wodma():
        nc.gpsimd.dma_start(sel[:], sel_ap[:, :])
        for kc in range(N_KCHUNK):
            nc.gpsimd.dma_start(wo[kc][:], wo_ap[ts(kc, P), :])

    # ---- filler units: generators yielding every ~2 matmuls ----
    def qk_unit(sc, w_list, o_list, m):
        def gen():
            pc = psD.tile([P, 512], F32, tag="pd")
            for dc in range(N_DCHUNK):
                nc.tensor.matmul(
                    pc[:],
                    lhsT=w_list[dc][:, ts(m, P)],
                    rhs=xT[dc][sc][:],
                    start=(dc == 0),
                    stop=(dc == N_DCHUNK - 1),
                )
                if dc < N_DCHUNK - 1:
                    yield
            nc.vector.tensor_copy(o_list[m][:, ts(sc, 512)], pc[:])
        return gen

    def v_unit(sc, j):
        def gen():
            st = sc * 4 + j
            pc = psD.tile([P, 512], F32, tag="pd")
            for dc in range(N_DCHUNK):
                nc.tensor.matmul(
                    pc[:],
                    lhsT=xT[dc][sc][:, ts(j, P)],
                    rhs=wv[dc][:],
                    start=(dc == 0),
                    stop=(dc == N_DCHUNK - 1),
                )
                if dc < N_DCHUNK - 1:
                    yield
            nc.vector.tensor_copy(
                vaug[st][:, :, 0:HD],
                pc[:].rearrange("p (h d) -> p h d", h=NH),
            )
            nc.gpsimd.memset(vaug[st][:, :, HD : HD + 1], 1.0)
        return gen

    def proj_unit(st):
        # yields after every matmul: fine-grained chunks so the feed supply
        # can be rationed across all of the last attention chunk's packs
        def gen():
            ot = o_pool.tile([P, D], F32, tag="o")
            for ncol in range(2):
                po = psD.tile([P, 512], F32, tag="pd")
                for kc in range(N_KCHUNK):
                    nc.tensor.matmul(
                        po[:],
                        lhsT=yT[kc][:, ts(st, P)],
                        rhs=wo[kc][:, ts(ncol, 512)],
                        start=(kc == 0),
                        stop=(kc == N_KCHUNK - 1),
                    )
                    if kc < N_KCHUNK - 1:
                        yield
                nc.vector.tensor_copy(ot[:, ts(ncol, 512)], po[:])
                if ncol == 0:
                    yield
            nc.sync.dma_start(out_ap[ts(st, P), :], ot[:])
        return gen

    def qkv_units(sc, q_first=False):
        units = []
        if q_first:
            # all q before k: lets chunk-0 projections start as soon as
            # x chunk 0 + wq have landed, before wk/wv arrive
            for m in range(N_KCHUNK):
                units.append(qk_unit(sc, wq, qT, m))
            for m in range(N_KCHUNK):
                units.append(qk_unit(sc, wk, kT, m))
        else:
            for m in range(N_KCHUNK):
                units.append(qk_unit(sc, wq, qT, m))
                units.append(qk_unit(sc, wk, kT, m))
        for j in range(4):
            units.append(v_unit(sc, j))
        return units

    feed = Feed()

    # ---- attention for one q-chunk, dense work fed between packs ----
    def emit_attn(qc, units=(), nres=0):
        units = list(units)
        if nres:
            feed.add(units[:-nres])
        else:
            feed.add(units)
        q0 = qc * 512
        n_kt = qc * 4 + 4
        diag0 = qc * 4
        dall = r_pool.tile([1, NH * 512], F32, tag="da")
        yts = []
        for h in range(NH):
            tile_i = h // 2
            row0 = (h % 2) * HD
            kT_h = kT[tile_i][row0 : row0 + HD, :]
            qT_h = qT[tile_i][row0 : row0 + HD, :]
            psum_y = psY.tile([P, 512], F32, tag="py", name=f"py{qc}_{h}")
            prev = None
            for p0 in range(0, n_kt, 2):
                pack = [p0, p0 + 1]
                pss = psS.tile([P, 1024], F32, tag="ps", name=f"ps{qc}_{h}_{p0}")
                pT = pT_pool.tile([P, 1024], BF16, tag="pT")
                offs = {}
                for idx, kt in enumerate(pack):
                    w = 512 if kt < diag0 else 512 - 128 * (kt - diag0)
                    off = idx * 512
                    qoff = q0 + (512 - w)
                    nc.tensor.matmul(
                        pss[:, ds(off, w)],
                        lhsT=kT_h[:, ts(kt, P)],
                        rhs=qT_h[:, ds(qoff, w)],
                        start=True,
                        stop=True,
                    )
                    offs[kt] = (off, w)
                # exp over contiguous runs
                runs = []
                for kt in pack:
                    off, w = offs[kt]
                    if runs and runs[-1][1] == off:
                        runs[-1][1] = off + w
                    else:
                        runs.append([off, off + w])
                for r0, r1 in runs:
                    nc.scalar.activation(
                        pT[:, ds(r0, r1 - r0)], pss[:, ds(r0, r1 - r0)], EXP, scale=SCALE
                    )
                for kt in pack:
                    off, w = offs[kt]
                    if kt >= diag0:
                        nc.gpsimd.tensor_mul(
                            pT[:, ds(off, P)], pT[:, ds(off, P)], trimask[:]
                        )
                if prev is not None:
                    for pkt, ppT, poff, pw in prev:
                        nc.tensor.matmul(
                            psum_y[0 : HD + 1, ds(512 - pw, pw)],
                            lhsT=vaug[pkt][:, h, :],
                            rhs=ppT[:, ds(poff, pw)],
                            start=(pkt == 0),
                            stop=False,
                            skip_group_check=True,
                        )
                prev = [(kt, pT, offs[kt][0], offs[kt][1]) for kt in pack]
                feed.feed()
            for i, (pkt, ppT, poff, pw) in enumerate(prev):
                nc.tensor.matmul(
                    psum_y[0 : HD + 1, ds(512 - pw, pw)],
                    lhsT=vaug[pkt][:, h, :],
                    rhs=ppT[:, ds(poff, pw)],
                    start=(pkt == 0),
                    stop=(pkt == n_kt - 1),
                    skip_group_check=True,
                )
            if h % 2 == 0:
                ytp = ytmp_pool.tile([P, 512], BF16, tag="yt")
                yts.append(ytp)
            else:
                ytp = yts[-1]
            nc.vector.tensor_copy(ytp[ds(row0, HD), :], psum_y[0:HD, :])
            # den row (PSUM partition 64) -> strip segment (partition 0)
            nc.vector.tensor_copy(dall[0:1, ds(h * 512, 512)], psum_y[HD : HD + 1, :])
            feed.feed()
        # repartition [1,4096] -> [8,512] via DRAM bounce, batch the reciprocal
        dscr = dram_pool.tile([NH * 512], F32, tag="ds")
        nc.sync.dma_start(dscr[:], dall[0:1, :])
        stage = r_pool.tile([NH, 512], F32, tag="st")
        nc.sync.dma_start(stage[:], dscr[:].rearrange("(p c) -> p c", p=NH))
        rec = r_pool.tile([NH, 512], F32, tag="rc")
        nc.vector.reciprocal_approx_fast(rec[:], stage[:])
        rbf = r_pool.tile([NH, 512], BF16, tag="rb")
        nc.gpsimd.tensor_copy(rbf[:], rec[:])
        if nres:
            feed.add(units[-nres:])
        feed.feed()
        feed.feed()
        for hp in range(NH // 2):
            # one K=8 selector matmul broadcasts recip rows for a head PAIR:
            # psr rows 0:64 = head 2*hp, rows 64:128 = head 2*hp+1 (matches
            # the yt pair tile layout), so a single [128,512] multiply
            # normalizes both heads at once
            psr = psY.tile([P, 512], F32, tag="py", name=f"pr{qc}_{hp}")
            nc.tensor.matmul(
                psr[:], lhsT=sel[:, ds(2 * hp * HD, P)], rhs=rbf[:], start=True, stop=True
            )
            rfull = r_pool.tile([P, 512], BF16, tag="rf")
            nc.vector.tensor_copy(rfull[:], psr[:])
            nc.vector.tensor_mul(yT[hp][:, ts(qc, 512)], yts[hp][:], rfull[:])
            feed.feed()

    # ---- emission schedule ----
    emit_xdma(0)
    emit_wdma()
    emit_xdma(1)
    for g in qkv_units(0, q_first=True):  # chunk 0 qkv inline (nothing to overlap)
        for _ in g():
            pass
    emit_wodma()
    emit_xdma(2)

    emit_attn(0, qkv_units(1), nres=2)
    feed.drain()               # qkv(1) must complete before attn(1)
    emit_xdma(3)

    emit_attn(1, qkv_units(2), nres=2)
    feed.drain()

    emit_attn(2, qkv_units(3), nres=2)
    feed.drain()

    emit_attn(3, [proj_unit(st) for st in range(0, 12)], nres=4)
    feed.drain()

    for g in [proj_unit(st) for st in range(12, 16)]:
        for _ in g():
            pass


def build_nc():
    nc = bacc.Bacc("TRN2", target_bir_lowering=False, debug=False)
    x_ap = nc.dram_tensor("x", [S, D], BF16, kind="ExternalInput").ap()
    wq_ap = nc.dram_tensor("wq", [D, DSH], BF16, kind="ExternalInput").ap()
    wk_ap = nc.dram_tensor("wk", [D, DSH], BF16, kind="ExternalInput").ap()
    wv_ap = nc.dram_tensor("wv", [D, DSH], BF16, kind="ExternalInput").ap()
    wo_ap = nc.dram_tensor("wo", [DSH, D], BF16, kind="ExternalInput").ap()
    sel_ap = nc.dram_tensor("sel", [NH, NH * HD], BF16, kind="ExternalInput").ap()
    out_ap = nc.dram_tensor("out", [S, D], F32, kind="ExternalOutput").ap()
    with tile.TileContext(nc) as tc:
        with ExitStack() as ctx:
            _emit(ctx, tc, x_ap, wq_ap, wk_ap, wv_ap, wo_ap, sel_ap, out_ap)
    nc.compile()
    return nc


_NC = None


def _get_nc():
    global _NC
    if _NC is None:
        _NC = build_nc()
    return _NC


def _bf16(a):
    import ml_dtypes

    return np.ascontiguousarray(a.astype(ml_dtypes.bfloat16))


def make_in_maps(x, Wqkv, Wo):
    Wq, Wk, Wv = Wqkv[:, 0:D], Wqkv[:, D : 2 * D], Wqkv[:, 2 * D : 3 * D]
    sel = np.kron(np.eye(NH, dtype=np.float32), np.ones((1, HD), np.float32))
    in_maps = []
    for c in range(8):
        b, hh = c // 2, c % 2
        cs = slice(hh * DSH, (hh + 1) * DSH)
        in_maps.append(
            {
                "x": _bf16(x[b]),
                "wq": _bf16(Wq[:, cs]),
                "wk": _bf16(Wk[:, cs]),
                "wv": _bf16(Wv[:, cs]),
                "wo": _bf16(Wo[cs, :]),
                "sel": _bf16(sel),
            }
        )
    return in_maps


def kernel(x, Wqkv, Wo, trace=False):
    x = np.asarray(x)
    Wqkv = np.asarray(Wqkv)
    Wo = np.asarray(Wo)
    nc = _get_nc()
    res = run_bass_kernel_spmd(nc, make_in_maps(x, Wqkv, Wo), list(range(8)), trace=trace)
    out = np.empty((4, S, D), np.float32)
    for b in range(4):
        out[b] = res.results[2 * b]["out"] + res.results[2 * b + 1]["out"]
    if trace:
        kernel.last_exec_time_ns = res.exec_time_ns
        kernel.last_results = res
    return out


# revision 25
# speedup vs baseline: 1.2663x; 1.2256x over previous
"""Causal multi-head attention (B=4, S=2048, D=1024, H=16, hd=64) on 8 TRN2 cores.

Sharding: core c handles batch b = c//2 and heads [8*(c%2), 8*(c%2)+8).
Each core computes a partial output y_h @ Wo_rows for its 8 heads over its
batch; the host sums the two partials per batch.

v3 strategy (per core):
  - Host pre-casts x/Wqkv/Wo to bf16 (identical numerics to the previous
    on-chip casts since every consumer used bf16) -> input DMA halves.
  - xT loaded directly transposed from DRAM via DMA XBAR transpose
    (dma_start_transpose), one contiguous [128,512] tile per (dc, chunk):
    no PE transposes, no PSUM staging, no DVE copies for x at all.
  - qT/kT computed directly transposed (W-chunk stationary, xT moving);
    v natural with an appended ones column per head (v_aug) so PV also
    yields softmax denominators.
  - Scores transposed sT[k,q] = K @ Q^T, k-tiles in packs of 2 -> one
    2-bank PSUM strip, exp on ACT over contiguous runs (scale=1/8 folded
    in), causal triangle = 0/1 multiply (GpSimd) on diagonal blocks only.
  - Software pipelining: PV(pack i) emitted after scores(pack i+1), and
    dense qkv/proj matmul work is fed in ~2-matmul micro-chunks between
    packs (attention alone is ACT-bound; the in-order PE queue needs ready
    dense work at sub-head granularity).
  - Softmax reciprocal: each head's denominator row (PSUM partition 64) is
    copied into a [1,4096] strip (GpSimd), one SBUF->SBUF DMA repartitions
    it to [8,512], one DVE reciprocal_approx_fast, then a K=8 selector
    matmul per head broadcasts row h across 64 partitions. (Replaces the
    old per-head 4-DMA DRAM bounce.)
  - PSUM: 8 banks = psS(scores 2-bank)x2 + psY(y/bcast)x2 + psD(qkv/proj)x2.
"""

import numpy as np
from contextlib import ExitStack

import concourse.bass as bass
import concourse.tile as tile
from concourse import bacc, mybir
from concourse.bass import ts, ds
from concourse.bass_utils import run_bass_kernel_spmd
from concourse.masks import make_upper_triangular

S = 2048
D = 1024
NH = 8          # heads per core
HD = 64         # head dim
DSH = NH * HD   # 512, per-core shard width
P = 128
F32 = mybir.dt.float32
BF16 = mybir.dt.bfloat16
EXP = mybir.ActivationFunctionType.Exp
SCALE = 1.0 / 8.0  # 1/sqrt(HD)

N_STILES = S // P        # 16
N_QCHUNK = S // 512      # 4
N_DCHUNK = D // P        # 8
N_KCHUNK = DSH // P      # 4


class Feed:
    """Round-robin micro-chunk scheduler over generator work units."""

    def __init__(self):
        self.queue = []
        self.cur = None

    def add(self, gens):
        self.queue.extend(gens)

    def feed(self):
        while True:
            if self.cur is None:
                if not self.queue:
                    return False
                self.cur = self.queue.pop(0)()
            try:
                next(self.cur)
                return True
            except StopIteration:
                self.cur = None

    def drain(self):
        while self.feed():
            pass


def _emit(ctx: ExitStack, tc: tile.TileContext, x_ap, wq_ap, wk_ap, wv_ap, wo_ap, sel_ap, out_ap):
    nc = tc.nc

    const = ctx.enter_context(tc.tile_pool(name="const", bufs=1))
    trimask = const.tile([P, P], BF16, tag="trimask")
    make_upper_triangular(nc, trimask, val=1.0, diag=True)
    # sel[i, h*64+j] = (i == h): K=8 selector that broadcasts recip row h
    # across 64 partitions in one matmul. Loaded from host.
    sel = const.tile([NH, NH * HD], BF16, tag="sel")

    xT_pool = ctx.enter_context(tc.tile_pool(name="xT", bufs=1))
    w_pool = ctx.enter_context(tc.tile_pool(name="w", bufs=1))

    # xT[dc][sc]: contiguous [128, 512] tiles (XBAR-transposed loads need a
    # contiguous SBUF destination)
    xT = [
        [xT_pool.tile([P, 512], BF16, tag=f"xT{dc}_{sc}", name=f"xT{dc}_{sc}") for sc in range(4)]
        for dc in range(N_DCHUNK)
    ]
    wq = [w_pool.tile([P, DSH], BF16, tag=f"wq{dc}", name=f"wq{dc}") for dc in range(N_DCHUNK)]
    wk = [w_pool.tile([P, DSH], BF16, tag=f"wk{dc}", name=f"wk{dc}") for dc in range(N_DCHUNK)]
    wv = [w_pool.tile([P, DSH], BF16, tag=f"wv{dc}", name=f"wv{dc}") for dc in range(N_DCHUNK)]
    wo = [w_pool.tile([P, D], BF16, tag=f"wo{kc}", name=f"wo{kc}") for kc in range(N_KCHUNK)]

    qkT_pool = ctx.enter_context(tc.tile_pool(name="qkT", bufs=1))
    qT = [qkT_pool.tile([P, S], BF16, tag=f"qT{m}", name=f"qT{m}") for m in range(N_KCHUNK)]
    kT = [qkT_pool.tile([P, S], BF16, tag=f"kT{m}", name=f"kT{m}") for m in range(N_KCHUNK)]
    vaug_pool = ctx.enter_context(tc.tile_pool(name="vaug", bufs=1))
    vaug = [vaug_pool.tile([P, NH, HD + 1], BF16, tag=f"v{st}", name=f"v{st}") for st in range(N_STILES)]
    yT_pool = ctx.enter_context(tc.tile_pool(name="yTp", bufs=1))
    yT = [yT_pool.tile([P, S], BF16, tag=f"yT{kc}", name=f"yT{kc}") for kc in range(N_KCHUNK)]

    pT_pool = ctx.enter_context(tc.tile_pool(name="pT", bufs=3))
    ytmp_pool = ctx.enter_context(tc.tile_pool(name="ytp", bufs=5))
    r_pool = ctx.enter_context(tc.tile_pool(name="rp", bufs=2))
    o_pool = ctx.enter_context(tc.tile_pool(name="op", bufs=3))

    dram_pool = ctx.enter_context(tc.tile_pool(name="drp", bufs=2, space="DRAM"))
    psS = ctx.enter_context(tc.tile_pool(name="psS", bufs=2, space="PSUM"))
    psY = ctx.enter_context(tc.tile_pool(name="psY", bufs=2, space="PSUM"))
    psD = ctx.enter_context(tc.tile_pool(name="psD", bufs=2, space="PSUM"))

    def emit_xdma(sc):
        for dc in range(N_DCHUNK):
            nc.sync.dma_start_transpose(
                xT[dc][sc][:], x_ap[ds(sc * 512, 512), ts(dc, P)]
            )

    def emit_wdma():
        for w_list, w_ap in ((wq, wq_ap), (wk, wk_ap), (wv, wv_ap)):
            for dc in range(N_DCHUNK):
                nc.sync.dma_start(w_list[dc][:], w_ap[ts(dc, P), :])

    def emit_wodma():
        nc.sync.dma_start(sel[:], sel_ap[:, :])
        for kc in range(N_KCHUNK):
            nc.sync.dma_start(wo[kc][:], wo_ap[ts(kc, P), :])

    # ---- filler units: generators yielding every ~2 matmuls ----
    def qk_unit(sc, w_list, o_list, m):
        def gen():
            pc = psD.tile([P, 512], F32, tag="pd")
            for dc in range(N_DCHUNK):
                nc.tensor.matmul(
                    pc[:],
                    lhsT=w_list[dc][:, ts(m, P)],
                    rhs=xT[dc][sc][:],
                    start=(dc == 0),
                    stop=(dc == N_DCHUNK - 1),
                )
                if dc < N_DCHUNK - 1:
                    yield
            nc.vector.tensor_copy(o_list[m][:, ts(sc, 512)], pc[:])
        return gen

    def v_unit(sc, j):
        def gen():
            st = sc * 4 + j
            pc = psD.tile([P, 512], F32, tag="pd")
            for dc in range(N_DCHUNK):
                nc.tensor.matmul(
                    pc[:],
                    lhsT=xT[dc][sc][:, ts(j, P)],
                    rhs=wv[dc][:],
                    start=(dc == 0),
                    stop=(dc == N_DCHUNK - 1),
                )
                if dc < N_DCHUNK - 1:
                    yield
            nc.vector.tensor_copy(
                vaug[st][:, :, 0:HD],
                pc[:].rearrange("p (h d) -> p h d", h=NH),
            )
            nc.gpsimd.memset(vaug[st][:, :, HD : HD + 1], 1.0)
        return gen

    def proj_unit(st):
        # yields after every matmul: fine-grained chunks so the feed supply
        # can be rationed across all of the last attention chunk's packs
        def gen():
            ot = o_pool.tile([P, D], F32, tag="o")
            for ncol in range(2):
                po = psD.tile([P, 512], F32, tag="pd")
                for kc in range(N_KCHUNK):
                    nc.tensor.matmul(
                        po[:],
                        lhsT=yT[kc][:, ts(st, P)],
                        rhs=wo[kc][:, ts(ncol, 512)],
                        start=(kc == 0),
                        stop=(kc == N_KCHUNK - 1),
                    )
                    if kc < N_KCHUNK - 1:
                        yield
                nc.vector.tensor_copy(ot[:, ts(ncol, 512)], po[:])
                if ncol == 0:
                    yield
            nc.sync.dma_start(out_ap[ts(st, P), :], ot[:])
        return gen

    def qkv_units(sc, q_first=False):
        units = []
        if q_first:
            # all q before k: lets chunk-0 projections start as soon as
            # x chunk 0 + wq have landed, before wk/wv arrive
            for m in range(N_KCHUNK):
                units.append(qk_unit(sc, wq, qT, m))
            for m in range(N_KCHUNK):
                units.append(qk_unit(sc, wk, kT, m))
        else:
            for m in range(N_KCHUNK):
                units.append(qk_unit(sc, wq, qT, m))
                units.append(qk_unit(sc, wk, kT, m))
        for j in range(4):
            units.append(v_unit(sc, j))
        return units

    feed = Feed()

    # ---- attention for one q-chunk, dense work fed between packs ----
    def emit_attn(qc, units=(), nres=0):
        units = list(units)
        if nres:
            feed.add(units[:-nres])
        else:
            feed.add(units)
        q0 = qc * 512
        n_kt = qc * 4 + 4
        diag0 = qc * 4
        dall = r_pool.tile([1, NH * 512], F32, tag="da")
        yts = []
        for h in range(NH):
            tile_i = h // 2
            row0 = (h % 2) * HD
            kT_h = kT[tile_i][row0 : row0 + HD, :]
            qT_h = qT[tile_i][row0 : row0 + HD, :]
            psum_y = psY.tile([P, 512], F32, tag="py", name=f"py{qc}_{h}")
            prev = None
            for p0 in range(0, n_kt, 2):
                pack = [p0, p0 + 1]
                pss = psS.tile([P, 1024], F32, tag="ps", name=f"ps{qc}_{h}_{p0}")
                pT = pT_pool.tile([P, 1024], BF16, tag="pT")
                offs = {}
                for idx, kt in enumerate(pack):
                    w = 512 if kt < diag0 else 512 - 128 * (kt - diag0)
                    off = idx * 512
                    qoff = q0 + (512 - w)
                    nc.tensor.matmul(
                        pss[:, ds(off, w)],
                        lhsT=kT_h[:, ts(kt, P)],
                        rhs=qT_h[:, ds(qoff, w)],
                        start=True,
                        stop=True,
                    )
                    offs[kt] = (off, w)
                # exp over contiguous runs
                runs = []
                for kt in pack:
                    off, w = offs[kt]
                    if runs and runs[-1][1] == off:
                        runs[-1][1] = off + w
                    else:
                        runs.append([off, off + w])
                for r0, r1 in runs:
                    nc.scalar.activation(
                        pT[:, ds(r0, r1 - r0)], pss[:, ds(r0, r1 - r0)], EXP, scale=SCALE
                    )
                for kt in pack:
                    off, w = offs[kt]
                    if kt >= diag0:
                        nc.gpsimd.tensor_mul(
                            pT[:, ds(off, P)], pT[:, ds(off, P)], trimask[:]
                        )
                if prev is not None:
                    for pkt, ppT, poff, pw in prev:
                        nc.tensor.matmul(
                            psum_y[0 : HD + 1, ds(512 - pw, pw)],
                            lhsT=vaug[pkt][:, h, :],
                            rhs=ppT[:, ds(poff, pw)],
                            start=(pkt == 0),
                            stop=False,
                            skip_group_check=True,
                        )
                prev = [(kt, pT, offs[kt][0], offs[kt][1]) for kt in pack]
                feed.feed()
            for i, (pkt, ppT, poff, pw) in enumerate(prev):
                nc.tensor.matmul(
                    psum_y[0 : HD + 1, ds(512 - pw, pw)],
                    lhsT=vaug[pkt][:, h, :],
                    rhs=ppT[:, ds(poff, pw)],
                    start=(pkt == 0),
                    stop=(pkt == n_kt - 1),
                    skip_group_check=True,
                )
            if h % 2 == 0:
                ytp = ytmp_pool.tile([P, 512], BF16, tag="yt")
                yts.append(ytp)
            else:
                ytp = yts[-1]
            nc.vector.tensor_copy(ytp[ds(row0, HD), :], psum_y[0:HD, :])
            # den row (PSUM partition 64) -> strip segment (partition 0)
            nc.vector.tensor_copy(dall[0:1, ds(h * 512, 512)], psum_y[HD : HD + 1, :])
            feed.feed()
        # repartition [1,4096] -> [8,512] via DRAM bounce, batch the reciprocal
        dscr = dram_pool.tile([NH * 512], F32, tag="ds")
        nc.sync.dma_start(dscr[:], dall[0:1, :])
        stage = r_pool.tile([NH, 512], F32, tag="st")
        nc.sync.dma_start(stage[:], dscr[:].rearrange("(p c) -> p c", p=NH))
        rec = r_pool.tile([NH, 512], F32, tag="rc")
        nc.vector.reciprocal_approx_fast(rec[:], stage[:])
        rbf = r_pool.tile([NH, 512], BF16, tag="rb")
        nc.gpsimd.tensor_copy(rbf[:], rec[:])
        if nres:
            feed.add(units[-nres:])
        feed.feed()
        feed.feed()
        for hp in range(NH // 2):
            # one K=8 selector matmul broadcasts recip rows for a head PAIR:
            # psr rows 0:64 = head 2*hp, rows 64:128 = head 2*hp+1 (matches
            # the yt pair tile layout), so a single [128,512] multiply
            # normalizes both heads at once
            psr = psY.tile([P, 512], F32, tag="py", name=f"pr{qc}_{hp}")
            nc.tensor.matmul(
                psr[:], lhsT=sel[:, ds(2 * hp * HD, P)], rhs=rbf[:], start=True, stop=True
            )
            rfull = r_pool.tile([P, 512], BF16, tag="rf")
            nc.vector.tensor_copy(rfull[:], psr[:])
            nc.vector.tensor_mul(yT[hp][:, ts(qc, 512)], yts[hp][:], rfull[:])
            feed.feed()

    # ---- emission schedule ----
    emit_xdma(0)
    emit_wdma()
    emit_xdma(1)
    for g in qkv_units(0, q_first=True):  # chunk 0 qkv inline (nothing to overlap)
        for _ in g():
            pass
    emit_wodma()
    emit_xdma(2)

    emit_attn(0, qkv_units(1), nres=2)
    feed.drain()               # qkv(1) must complete before attn(1)
    emit_xdma(3)

    emit_attn(1, qkv_units(2), nres=2)
    feed.drain()

    emit_attn(2, qkv_units(3), nres=2)
    feed.drain()

    emit_attn(3, [proj_unit(st) for st in range(0, 12)], nres=4)
    feed.drain()

    for g in [proj_unit(st) for st in range(12, 16)]:
        for _ in g():
            pass


def build_nc():
    nc = bacc.Bacc("TRN2", target_bir_lowering=False, debug=False)
    x_ap = nc.dram_tensor("x", [S, D], BF16, kind="ExternalInput").ap()
    wq_ap = nc.dram_tensor("wq", [D, DSH], BF16, kind="ExternalInput").ap()
    wk_ap = nc.dram_tensor("wk", [D, DSH], BF16, kind="ExternalInput").ap()
    wv_ap = nc.dram_tensor("wv", [D, DSH], BF16, kind="ExternalInput").ap()
    wo_ap = nc.dram_tensor("wo", [DSH, D], BF16, kind="ExternalInput").ap()
    sel_ap = nc.dram_tensor("sel", [NH, NH * HD], BF16, kind="ExternalInput").ap()
    out_ap = nc.dram_tensor("out", [S, D], F32, kind="ExternalOutput").ap()
    with tile.TileContext(nc) as tc:
        with ExitStack() as ctx:
            _emit(ctx, tc, x_ap, wq_ap, wk_ap, wv_ap, wo_ap, sel_ap, out_ap)
    nc.compile()
    return nc


_NC = None


def _get_nc():
    global _NC
    if _NC is None:
        _NC = build_nc()
    return _NC


def _bf16(a):
    import ml_dtypes

    return np.ascontiguousarray(a.astype(ml_dtypes.bfloat16))


def make_in_maps(x, Wqkv, Wo):
    Wq, Wk, Wv = Wqkv[:, 0:D], Wqkv[:, D : 2 * D], Wqkv[:, 2 * D : 3 * D]
    sel = np.kron(np.eye(NH, dtype=np.float32), np.ones((1, HD), np.float32))
    in_maps = []
    for c in range(8):
        b, hh = c // 2, c % 2
        cs = slice(hh * DSH, (hh + 1) * DSH)
        in_maps.append(
            {
                "x": _bf16(x[b]),
                "wq": _bf16(Wq[:, cs]),
                "wk": _bf16(Wk[:, cs]),
                "wv": _bf16(Wv[:, cs]),
                "wo": _bf16(Wo[cs, :]),
                "sel": _bf16(sel),
            }
        )
    return in_maps


def kernel(x, Wqkv, Wo, trace=False):
    x = np.asarray(x)
    Wqkv = np.asarray(Wqkv)
    Wo = np.asarray(Wo)
    nc = _get_nc()
    res = run_bass_kernel_spmd(nc, make_in_maps(x, Wqkv, Wo), list(range(8)), trace=trace)
    out = np.empty((4, S, D), np.float32)
    for b in range(4):
        out[b] = res.results[2 * b]["out"] + res.results[2 * b + 1]["out"]
    if trace:
        kernel.last_exec_time_ns = res.exec_time_ns
        kernel.last_results = res
    return out
